# revision 26
# baseline (speedup 1.0000x reference)
"""MultiHeadAttentionLayer (per-position head-mixing attention) as a Bass/Tile
kernel on 8 Trainium2 NeuronCores.

Sharding: pure data-parallel over the flattened batch*seq position axis
(N*L = 16384 positions -> 2048 per core).  The reference "attention" mixes
HEADS within each position (einsum nlhd,nled->nlhe), so positions are fully
independent and no collectives are needed.

Per-core pipeline (position-major, fp16 on chip, fp32 PSUM), software-
pipelined so tile t+1's projections sit ahead of tile t's out-projection in
the in-order PE stream:
  - q/k/v projections: PE matmuls, stationary = X^T chunk, moving = W^T
    (host pre-transposes/pre-tiles inputs + weights, biases via K=1 matmul)
  - head-mix logits  : custom fused DVE op (prefix-scan of Src0*Src1) with a
    step-0 output AP, so segment-end prefixes land compactly; one batched
    diff yields all 256 per-position logits
  - softmax over e   : ACT exp (no max-subtraction needed; logits are O(1))
  - ctx              : same scan trick over v's (d,e) stream per head
  - out projection   : PE transpose of ctx -> PE matmuls with Wo^T
"""

import sys

if "/opt/trn_rl_repo" not in sys.path:
    sys.path.insert(0, "/opt/trn_rl_repo")

import numpy as np

N, L, HID, EMB, NH, HD = 4, 4096, 1024, 1024, 16, 64
NCORES = 8
POS = N * L            # 16384 positions
PPC = POS // NCORES    # 2048 per core
PT = 128               # positions per tile
NT = PPC // PT         # 16 tiles per core
KS = 8                 # contraction chunks (1024 / 128)

_CACHE = {}


def _mul_scan_op():
    """Custom fused DVE op: out = inclusive prefix-sum of (in0 * in1).

    Segmented dot products then fall out as strided boundary differences,
    replacing (tensor_mul + tensor_reduce) pairs with one 1x-rate pass."""
    if "mul_scan" in _CACHE:
        return _CACHE["mul_scan"]
    import concourse.dve_ops as DO
    from concourse.dve_ops import DveOp
    from concourse.dve_spec import AluOp, Spec, Src0, Src1, scan

    def _ref(in0, in1, s0, s1, imm2):
        a = np.asarray(in0, dtype=np.float32)
        b = np.asarray(in1, dtype=np.float32)
        if b.size == a.size:
            b = b.reshape(a.shape)
        else:
            b = np.broadcast_to(b, a.shape)
        f = (a * b).reshape(a.shape[0], -1)
        return np.cumsum(f, axis=1).reshape(a.shape)

    op = DveOp(
        "MUL_SCAN_ANT",
        Spec(body=scan(AluOp.ADD, Src0 * Src1), reference=_ref),
        subdim=False,
        uops_sha={"v3": "b3fc3e78a862b7eb", "v4": "bc6a002865d48b97"},
    )
    DO.OPS.append(op)
    DO.CUSTOM_DVE_SPECS[op.name] = op.spec
    DO._SUB_OPCODE_FOR_NAME[op.name] = DO._CUSTOM_DVE_ROW_BASE + len(DO.OPS) - 1
    _CACHE["mul_scan"] = op
    return op


def build_program(nt=NT):
    """Build the (uncompiled) Bass/Tile program for nt position-tiles."""
    from contextlib import ExitStack

    import concourse.bacc as bacc
    import concourse.mybir as mybir
    from concourse import tile

    dt = mybir.dt
    F16, F32 = dt.float16, dt.float32
    AX = mybir.AxisListType
    AF = mybir.ActivationFunctionType

    nc = bacc.Bacc(
        "TRN2", target_bir_lowering=False, debug=False, num_devices=NCORES
    )

    # --- DRAM parameters (per-core) ---
    # activations pre-tiled on host: [nt, 128(hid-in-chunk), 8(chunk), 128(pos)]
    xt = nc.dram_tensor("xt", [nt, PT, KS, PT], F16, kind="ExternalInput").ap()
    kt = nc.dram_tensor("kt", [nt, PT, KS, PT], F16, kind="ExternalInput").ap()
    vt = nc.dram_tensor("vt", [nt, PT, KS, PT], F16, kind="ExternalInput").ap()
    # weights pre-tiled on host: [128(hid-in-chunk), 8(chunk), 1024(emb out)]
    wq = nc.dram_tensor("wq", [PT, KS, EMB], F16, kind="ExternalInput").ap()
    wk = nc.dram_tensor("wk", [PT, KS, EMB], F16, kind="ExternalInput").ap()
    wv = nc.dram_tensor("wv", [PT, KS, EMB], F16, kind="ExternalInput").ap()
    wo = nc.dram_tensor("wo", [PT, KS, HID], F16, kind="ExternalInput").ap()
    bq = nc.dram_tensor("bq", [1, EMB], F16, kind="ExternalInput").ap()
    bk = nc.dram_tensor("bk", [1, EMB], F16, kind="ExternalInput").ap()
    bv = nc.dram_tensor("bv", [1, EMB], F16, kind="ExternalInput").ap()
    bo = nc.dram_tensor("bo", [1, HID], F16, kind="ExternalInput").ap()
    ident = nc.dram_tensor("ident", [PT, PT], F16, kind="ExternalInput").ap()
    out = nc.dram_tensor("out", [nt * PT, HID], F16, kind="ExternalOutput").ap()

    with tile.TileContext(nc) as tc, ExitStack() as ctx:
        const = ctx.enter_context(tc.tile_pool(name="const", bufs=1))
        work = ctx.enter_context(tc.tile_pool(name="work", bufs=4))
        psum = ctx.enter_context(tc.tile_pool(name="psum", bufs=3, space="PSUM"))
        pstp = ctx.enter_context(tc.tile_pool(name="pst", bufs=2, space="PSUM"))

        # resident constants
        w_sb = {}
        for nm, drm in (("wq", wq), ("wk", wk), ("wv", wv), ("wo", wo)):
            t = const.tile([PT, KS, EMB], F16, tag=nm)
            nc.sync.dma_start(t[:], drm)
            w_sb[nm] = t
        b_sb = {}
        for nm, drm in (("bq", bq), ("bk", bk), ("bv", bv), ("bo", bo)):
            t = const.tile([1, EMB], F16, tag=nm)
            nc.sync.dma_start(t[:], drm)
            b_sb[nm] = t
        id_sb = const.tile([PT, PT], F16, tag="ident")
        nc.sync.dma_start(id_sb[:], ident)
        ones = const.tile([1, PT], F16, tag="ones")
        nc.gpsimd.memset(ones[:], 1.0)

        def proj_tile(t):
            """Loads + q/k/v projections for tile t -> dict of SBUF tiles."""
            ins = {}
            for nm, drm in (("x", xt), ("k", kt), ("v", vt)):
                tl = work.tile([PT, KS, PT], F16, tag=nm + "in")
                nc.sync.dma_start(tl[:], drm[t])
                ins[nm] = tl
            sb = {}
            for nm, src, wn, bn in (
                ("q", "x", "wq", "bq"),
                ("k", "k", "wk", "bk"),
                ("v", "v", "wv", "bv"),
            ):
                ps = psum.tile([PT, EMB], F32, tag="ps")
                for s in range(KS):
                    for h2 in (0, 1):
                        nc.tensor.matmul(
                            ps[:, h2 * 512 : (h2 + 1) * 512],
                            ins[src][:, s, :],
                            w_sb[wn][:, s, h2 * 512 : (h2 + 1) * 512],
                            start=(s == 0),
                            stop=False,
                        )
                for h2 in (0, 1):
                    nc.tensor.matmul(
                        ps[:, h2 * 512 : (h2 + 1) * 512],
                        ones[:],
                        b_sb[bn][:, h2 * 512 : (h2 + 1) * 512],
                        start=False,
                        stop=True,
                    )
                o = work.tile([PT, EMB], F16, tag=nm)
                nc.scalar.activation(o[:], ps[:], AF.Copy)
                sb[nm] = o
            return sb

        def attn_tile(sb):
            # ---- logits[p, h, e] = sum_d q[p,h,d] * k[p,e,d]  (q pre-scaled) ----
            # Per h: fused mul+prefix-scan over k's (e,d) stream with a
            # step-0 innermost OUT pattern — every prefix in a segment lands
            # on the same address, so the surviving value is the segment-end
            # prefix.  Each h writes its own row of `scanb` (col 0 pinned to
            # zero); one batched tensor_sub then produces all 256 logits.
            MS = _mul_scan_op()
            logits = work.tile([PT, NH, NH], F32, tag="logits")
            scanb = work.tile([PT, NH, 1 + NH], F32, tag="scanb")
            scanc = work.tile([PT, NH, 1 + HD], F32, tag="scanc")
            nc.vector.memset(scanb[:, :, 0:1], 0.0)
            nc.vector.memset(scanc[:, :, 0:1], 0.0)
            kv = sb["k"][:].rearrange("p (e d) -> p e d", d=HD)
            for h in range(NH):
                qsl = (
                    sb["q"][:, h * HD : (h + 1) * HD]
                    .unsqueeze(1)
                    .to_broadcast((PT, NH, HD))
                )
                nc.vector._custom_dve(
                    MS,
                    out=scanb[:, h, 1:].unsqueeze(2).to_broadcast((PT, NH, HD)),
                    in0=kv,
                    in1=qsl,
                )
            nc.vector.tensor_sub(
                logits[:], scanb[:, :, 1:], scanb[:, :, 0:NH]
            )

            # ---- softmax over e (no max subtraction: |logits| ~ O(1)) ----
            exps = work.tile([PT, NH, NH], F32, tag="exps")
            nc.scalar.activation(
                exps[:].rearrange("p h e -> p (h e)"),
                logits[:].rearrange("p h e -> p (h e)"),
                AF.Exp,
            )
            denom = work.tile([PT, NH], F32, tag="denom")
            nc.vector.reduce_sum(denom[:], exps[:], axis=AX.X)
            recip = work.tile([PT, NH], F32, tag="recip")
            nc.vector.reciprocal(recip[:], denom[:])
            attn = work.tile([PT, NH, NH], F16, tag="attn")
            nc.vector.tensor_mul(
                attn[:], exps[:], recip[:].unsqueeze(2).to_broadcast((PT, NH, NH))
            )

            # ---- ctx[p, d, h] = sum_e attn[p,h,e] * v[p,e,d] ----
            # v was projected with (d,e)-major column order, so sb["v"] is
            # [p,(d,e)].  Per h: fused mul+scan over the (d,e) stream, then
            # ctx[:,d,h] are the 16-segment boundary differences.
            ctxs = work.tile([PT, NH, HD], F16, tag="ctxs")  # (h, d)-major
            vv = sb["v"][:].rearrange("p (d e) -> p d e", e=NH)
            for h in range(NH):
                a_h = attn[:, h : h + 1, :].to_broadcast((PT, HD, NH))
                nc.vector._custom_dve(
                    MS,
                    out=scanc[:, h, 1:].unsqueeze(2).to_broadcast((PT, HD, NH)),
                    in0=vv,
                    in1=a_h,
                )
            nc.vector.tensor_sub(
                ctxs[:], scanc[:, :, 1:], scanc[:, :, 0:HD]
            )
            return ctxs

        def out_tile(t, ctxs):
            # ---- out = ctx @ Wo^T + bo  (PE transpose ctx -> matmuls) ----
            ctxf = ctxs[:].rearrange("p h d -> p (h d)")
            ctxT = work.tile([PT, KS, PT], F16, tag="ctxT")
            for c in range(KS):
                pst = pstp.tile([PT, PT], F16, tag="pst")
                nc.tensor.transpose(
                    pst[:], ctxf[:, c * PT : (c + 1) * PT], id_sb[:]
                )
                nc.scalar.activation(ctxT[:, c, :], pst[:], AF.Copy)

            ps2 = psum.tile([PT, HID], F32, tag="ps")
            for s in range(KS):
                for h2 in (0, 1):
                    nc.tensor.matmul(
                        ps2[:, h2 * 512 : (h2 + 1) * 512],
                        ctxT[:, s, :],
                        w_sb["wo"][:, s, h2 * 512 : (h2 + 1) * 512],
                        start=(s == 0),
                        stop=False,
                    )
            for h2 in (0, 1):
                nc.tensor.matmul(
                    ps2[:, h2 * 512 : (h2 + 1) * 512],
                    ones[:],
                    b_sb["bo"][:, h2 * 512 : (h2 + 1) * 512],
                    start=False,
                    stop=True,
                )
            o_sb = work.tile([PT, HID], F16, tag="osb")
            nc.scalar.activation(o_sb[:], ps2[:], AF.Copy)
            nc.sync.dma_start(out[t * PT : (t + 1) * PT, :], o_sb[:])

        # Software pipeline: emit tile t+1's projections (PE) before tile
        # t's out-projection so the in-order PE stream never stalls on the
        # DVE attention of the current tile.
        sb_next = proj_tile(0)
        for t in range(nt):
            sb_cur, sb_next = sb_next, None
            ctxs = attn_tile(sb_cur)
            if t + 1 < nt:
                sb_next = proj_tile(t + 1)
            out_tile(t, ctxs)

    nc.compile()
    return nc


def _tile_acts(a2d):
    """[PPC, HID] fp16 -> [NT, 128(hid-in-chunk), 8(chunk), 128(pos)]."""
    return np.ascontiguousarray(
        a2d.reshape(NT, PT, KS, PT).transpose(0, 3, 2, 1)
    )


def _tile_w(wT):
    """[HID, EMB] -> [128(hid-in-chunk), 8(chunk), EMB]."""
    return np.ascontiguousarray(wT.reshape(KS, PT, EMB).transpose(1, 0, 2))


def prep_inputs(Q, K, V, Wq, bq, Wk, bk, Wv, bv, Wo, bo, nt=NT):
    """Host-side prep: cast fp16, transpose/tile, shard across cores.

    Returns (in_maps, weight map is replicated)."""
    f16 = np.float16
    scale = np.float32(1.0 / np.sqrt(HD))

    # d-major permutation of the v/out feature axis: new_col j = d*NH+h maps
    # to old_col h*HD+d
    perm_de = (np.arange(EMB).reshape(NH, HD).T).reshape(-1)  # [d*16+e] -> e*64+d

    wq_t = _tile_w((Wq.T * scale).astype(f16))
    wk_t = _tile_w(Wk.T.astype(f16))
    wv_t = _tile_w(Wv.T[:, perm_de].astype(f16))  # v columns (d,e)-major
    wo_t = _tile_w(Wo.T.astype(f16))

    bq_t = (bq * scale).astype(f16).reshape(1, EMB)
    bk_t = bk.astype(f16).reshape(1, EMB)
    bv_t = bv.astype(f16)[perm_de].reshape(1, EMB)
    bo_t = bo.astype(f16).reshape(1, HID)
    id_t = np.eye(PT, dtype=f16)

    shared = dict(
        wq=wq_t, wk=wk_t, wv=wv_t, wo=wo_t,
        bq=bq_t, bk=bk_t, bv=bv_t, bo=bo_t, ident=id_t,
    )

    Qf = Q.reshape(POS, HID).astype(f16)
    Kf = K.reshape(POS, HID).astype(f16)
    Vf = V.reshape(POS, HID).astype(f16)

    in_maps = []
    npos = nt * PT
    for c in range(NCORES):
        s = c * PPC
        m = dict(shared)
        m["xt"] = _tile_acts_nt(Qf[s : s + npos], nt)
        m["kt"] = _tile_acts_nt(Kf[s : s + npos], nt)
        m["vt"] = _tile_acts_nt(Vf[s : s + npos], nt)
        in_maps.append(m)
    return in_maps


def _tile_acts_nt(a2d, nt):
    return np.ascontiguousarray(
        a2d.reshape(nt, PT, KS, PT).transpose(0, 3, 2, 1)
    )


def _get_runner():
    """Build program once and return a persistent jitted SPMD runner."""
    if "runner" in _CACHE:
        return _CACHE["runner"]

    import jax
    from jax.sharding import Mesh, PartitionSpec
    from jax.experimental.shard_map import shard_map
    from concourse import bass2jax
    import concourse.mybir as mybir

    nc = build_program(NT)
    bass2jax.install_neuronx_cc_hook()

    partition_name = (
        nc.partition_id_tensor.name if nc.partition_id_tensor else None
    )
    in_names = []
    out_names = []
    out_avals = []
    zero_outs = []
    for alloc in nc.m.functions[0].allocations:
        if not isinstance(alloc, mybir.MemoryLocationSet):
            continue
        if not alloc.memorylocations:
            continue
        name = alloc.memorylocations[0].name
        if alloc.kind == "ExternalInput":
            if name != partition_name:
                in_names.append(name)
        elif alloc.kind == "ExternalOutput":
            out_names.append(name)
            shape = tuple(alloc.tensor_shape)
            dtype = mybir.dt.np(alloc.dtype)
            out_avals.append(jax.core.ShapedArray(shape, dtype))
            zero_outs.append(np.zeros(shape, dtype))
    n_params = len(in_names)
    n_outs = len(out_avals)
    all_names = list(in_names) + list(out_names)
    if partition_name is not None:
        all_names.append(partition_name)

    def _body(*args):
        operands = list(args)
        if partition_name is not None:
            operands.append(bass2jax.partition_id_tensor())
        outs = bass2jax._bass_exec_p.bind(
            *operands,
            out_avals=tuple(out_avals),
            in_names=tuple(all_names),
            out_names=tuple(out_names),
            lowering_input_output_aliases=(),
            sim_require_finite=True,
            sim_require_nnan=True,
            nc=nc,
        )
        return tuple(outs)

    devices = jax.devices()[:NCORES]
    mesh = Mesh(np.asarray(devices), ("core",))
    specs = (PartitionSpec("core"),) * (n_params + n_outs)
    donate = tuple(range(n_params, n_params + n_outs))
    sharded = jax.jit(
        shard_map(
            _body,
            mesh=mesh,
            in_specs=specs,
            out_specs=(PartitionSpec("core"),) * n_outs,
            check_rep=False,
        ),
        donate_argnums=donate,
        keep_unused=True,
    )

    def run(in_maps):
        concat_in = [
            np.concatenate([in_maps[c][nm] for c in range(NCORES)], axis=0)
            for nm in in_names
        ]
        concat_zero = [
            np.zeros((NCORES * z.shape[0], *z.shape[1:]), z.dtype)
            for z in zero_outs
        ]
        out_arrs = sharded(*concat_in, *concat_zero)
        return [
            {
                nm: np.asarray(out_arrs[i]).reshape(
                    NCORES, *out_avals[i].shape
                )[c]
                for i, nm in enumerate(out_names)
            }
            for c in range(NCORES)
        ]

    _CACHE["runner"] = (nc, run)
    return _CACHE["runner"]


def assemble_output(results):
    full = np.concatenate(
        [results[c]["out"].astype(np.float32) for c in range(NCORES)], axis=0
    )
    return full.reshape(N, L, HID)


def kernel(Q, K, V, Wq, bq, Wk, bk, Wv, bv, Wo, bo):
    args = [np.asarray(a) for a in (Q, K, V, Wq, bq, Wk, bk, Wv, bv, Wo, bo)]
    in_maps = prep_inputs(*args)
    _, run = _get_runner()
    results = run(in_maps)
    return assemble_output(results)
